# revision 4
# baseline (speedup 1.0000x reference)
"""Trainium2 kernel v2 for the nn_Circuit coupled-mode ODE problem.

Math: dA/dt = i*diag(omega + gamma*|A|^2) A + T2 A, t in [0,2], 200 samples,
A is (1024 batch, 64 modes) complex padded with ones.  Strang splitting with
the linear part exact (host matrix exponentials) and the nonlinear part an
exact per-pair phase rotation, phase computed IN-STEP from |A|^2 (lagged
phases fail: the stiff ~288i eigenvalue makes |A|^2 swing ~25% per step).

Device formulation (interleaved re/im layout, partitions p=2j+c):
  state w_k (fp16 SBUF) at the rotation midpoint.  Per step:
    s2  = w (.) w                       (V, fp16 2x)
    m2  = P2 @ s2                       (PE; pair-sum, duplicated per pair)
    q   = sin(gh*m2 + bias)             (ACT; bias pi/2 even / 0 odd -> [cos;sin])
    q2  = pairswap(q)                   (V stream_shuffle -> [sin;cos])
    r1  = w (.) q,  r2 = w (.) q2       (V, fp16 2x)
    y_{k+1} = Y1 r1 + Y2 r2             (PE -> PSUM y-bank, DMA'd 4 steps at a time)
    w_ps    = W1 r1 + W2 r2             (PE; W split hi+lo fp16 for accuracy)
    w_{k+1} = copy(w_ps)                (V, PSUM->SBUF fp16 2x)
  where W1 = E(h) M1, W2 = E(h) M2, Y1 = E(h/2) M1, Y2 = E(h/2) M2 fold the
  complex-multiply pair-mixing (M1: re' = r1[2j]-r1[2j+1] -> even rows,
  M2: im' = r2[2j]+r2[2j+1] -> odd rows) into the matmul weights.

Two 64-column batch groups run the chain staggered so PE/V/ACT overlap across
groups; all weights fp16 (FWL fast weight load) so matmuls are single-pass.

Sharding: pure data parallel, batch 1024 = 8 cores x 128.
"""

import numpy as np

MODES = 64
INPUT_MODES = 48
BATCH = 1024
EVAL_PTS = 200
EPS = 1e-8
N_CORES = 8
B_LOC = BATCH // N_CORES  # 128
NT = EVAL_PTS - 1  # 199 intervals
DT = 2.0 / NT
NG = 2              # column groups per core
GW = B_LOC // NG    # 64 columns per group
TB = 8              # output time-steps batched per y-bank DMA

_CACHE = {}


# ---------------------------------------------------------------------------
# host-side math
# ---------------------------------------------------------------------------

def _t2_like_reference(params, omega, kappa):
    """Reproduce the reference's float32 jax computation of T2 exactly."""
    import jax

    try:
        cpu = jax.devices("cpu")[0]
    except Exception:
        cpu = None

    import contextlib

    ctx = jax.default_device(cpu) if cpu is not None else contextlib.nullcontext()
    with ctx:
        import jax.numpy as jnp

        n = MODES
        p = jnp.asarray(params, dtype=jnp.float32)
        n_off = n * (n - 1) // 2
        iu = jnp.triu_indices(n, 1)
        off = p[:n_off] + 1j * p[n_off:2 * n_off]
        H = jnp.zeros((n, n), dtype=jnp.complex64).at[iu].set(off.astype(jnp.complex64))
        H = H + H.conj().T
        d = p[2 * n_off:]
        diag = jnp.concatenate([d, -jnp.sum(d, keepdims=True)])
        H = H + jnp.diag(diag.astype(jnp.complex64))
        U = jax.scipy.linalg.expm(1j * H)
        I = jnp.eye(n, dtype=jnp.complex64)
        M = U.T @ U
        mix = M @ jnp.linalg.inv(I - M + EPS * I)
        T2 = -jnp.asarray(kappa, dtype=jnp.float32) * (
            0.5 * jnp.eye(n, dtype=jnp.float32) + mix
        )
        T2_re = np.asarray(jnp.real(T2), dtype=np.float32)
        T2_im = np.asarray(jnp.imag(T2), dtype=np.float32)
    return T2_re, T2_im


def _expm(M):
    w, V = np.linalg.eig(M)
    return (V * np.exp(w)) @ np.linalg.inv(V)


def _big_il(C):
    """Complex (64,64) -> real (128,128) operator in the interleaved re/im basis."""
    A = np.zeros((2 * MODES, 2 * MODES), dtype=np.float64)
    Cr, Ci = C.real, C.imag
    A[0::2, 0::2] = Cr
    A[0::2, 1::2] = -Ci
    A[1::2, 0::2] = Ci
    A[1::2, 1::2] = Cr
    return A


def _host_precompute(A0, params, omega, kappa, nonlinearity):
    T2_re, T2_im = _t2_like_reference(params, omega, kappa)
    L = T2_re.astype(np.float64) + 1j * T2_im.astype(np.float64)
    L = L + 1j * np.diag(omega.astype(np.float64))

    A1 = _big_il(_expm(L * DT))         # full-step propagator E(h)
    A2 = _big_il(_expm(L * (DT / 2)))   # half-step propagator E(h/2)

    # pair-mixing matrices (signs included)
    M1 = np.zeros((128, 128))
    M2 = np.zeros((128, 128))
    for j in range(MODES):
        M1[2 * j, 2 * j] = 1.0
        M1[2 * j, 2 * j + 1] = -1.0
        M2[2 * j + 1, 2 * j] = 1.0
        M2[2 * j + 1, 2 * j + 1] = 1.0

    W1 = A1 @ M1
    W2 = A1 @ M2
    Y1 = A2 @ M1
    Y2 = A2 @ M2

    def lhsT16(Wm):
        return np.ascontiguousarray(Wm.T, dtype=np.float16)

    def split16(Wm):
        hi = Wm.astype(np.float16).astype(np.float64)
        lo = Wm - hi
        return lhsT16(hi), lhsT16(lo)


    P2 = np.zeros((128, 128))
    for j in range(MODES):
        P2[2 * j, 2 * j] = P2[2 * j, 2 * j + 1] = 1.0
        P2[2 * j + 1, 2 * j] = P2[2 * j + 1, 2 * j + 1] = 1.0

    # initial state, interleaved mode-major: (128, BATCH)
    y0 = np.zeros((2 * MODES, BATCH), dtype=np.float64)
    y0[0:2 * INPUT_MODES:2, :] = A0[:, :, 0].astype(np.float64).T
    y0[1:2 * INPUT_MODES:2, :] = A0[:, :, 1].astype(np.float64).T
    y0[2 * INPUT_MODES::2, :] = 1.0
    w0 = (A2 @ y0).astype(np.float16)
    y0f = y0.astype(np.float32)

    gh = nonlinearity.astype(np.float64) * DT
    ghscale = np.repeat(gh, 2).astype(np.float32).reshape(128, 1)
    qbias = np.tile([np.pi / 2, 0.0], MODES).astype(np.float32).reshape(128, 1)

    W1hi, W1lo = split16(W1)
    W2hi, W2lo = split16(W2)
    return dict(
        W1hi=W1hi, W1lo=W1lo, W2hi=W2hi, W2lo=W2lo,
        Y1=lhsT16(Y1), Y2=lhsT16(Y2), P2T=lhsT16(P2.T),
        ghscale=ghscale, qbias=qbias, w0=w0, y0f=y0f,
    )


# ---------------------------------------------------------------------------
# device kernel
# ---------------------------------------------------------------------------

def _build_nc():
    import concourse.bass as bass
    import concourse.bacc as bacc
    import concourse.tile as tile
    import concourse.mybir as mybir

    f32 = mybir.dt.float32
    f16 = mybir.dt.float16
    Sin = mybir.ActivationFunctionType.Sin
    Square = mybir.ActivationFunctionType.Square
    mult = mybir.AluOpType.mult
    P = 128
    pairswap = [i ^ 1 for i in range(32)]

    nc = bacc.Bacc("TRN2", target_bir_lowering=False, debug=False,
                   num_devices=N_CORES)

    wts = {}
    for name in ("W1hi", "W1lo", "W2hi", "W2lo", "Y1", "Y2", "P2T"):
        wts[name] = nc.dram_tensor(name, [P, P], f16, kind="ExternalInput").ap()
    ghscale_d = nc.dram_tensor("ghscale", [P, 1], f32, kind="ExternalInput").ap()
    qbias_d = nc.dram_tensor("qbias", [P, 1], f32, kind="ExternalInput").ap()
    w0_d = nc.dram_tensor("w0", [P, B_LOC], f16, kind="ExternalInput").ap()
    # output t=1..199 (t=0 is assembled on the host), time-major per partition
    out_d = nc.dram_tensor("out", [P, EVAL_PTS - 1, B_LOC], f16,
                           kind="ExternalOutput").ap()

    with tile.TileContext(nc) as tc:
        with (
            tc.tile_pool(name="const", bufs=1) as cpool,
            tc.tile_pool(name="st", bufs=3) as spool,       # per-group SBUF work tiles
            tc.tile_pool(name="pw", bufs=2, space="PSUM") as pwpool,   # w_ps
            tc.tile_pool(name="pm", bufs=1, space="PSUM") as pmpool,   # m2
            tc.tile_pool(name="py", bufs=2, space="PSUM") as pypool,   # y step tiles
            tc.tile_pool(name="yb", bufs=2) as ybpool,                 # SBUF y banks
        ):
            wt = {}
            for name in ("W1hi", "W1lo", "W2hi", "W2lo", "Y1", "Y2", "P2T"):
                wt[name] = cpool.tile([P, P], f16, tag=name, name=name)
                nc.sync.dma_start(wt[name][:], wts[name][:])
            ghs_t = cpool.tile([P, 1], f32, tag="ghs")
            qb_t = cpool.tile([P, 1], f32, tag="qb")
            nc.sync.dma_start(ghs_t[:], ghscale_d[:])
            nc.sync.dma_start(qb_t[:], qbias_d[:])

            # initial chain state per group (SBUF; later steps read w from PSUM)
            w0_t = cpool.tile([P, B_LOC], f16, tag="w0t")
            nc.sync.dma_start(w0_t[:], w0_d[:])
            w_cur = [w0_t[:, g * GW:(g + 1) * GW] for g in range(NG)]

            ybanks = {}   # bank index -> (tile, pending y_ps slices)
            pend = []     # deferred (k, y_ps) staging copies

            def stage_pending():
                # emit the previous step's y staging AFTER this step's SINs so
                # the ACT queue never blocks the phase chain
                while pend:
                    pk, pyps = pend.pop(0)
                    b, ps = divmod(pk, TB)
                    if ps == 0:
                        ybanks[b] = ybpool.tile([P, TB * B_LOC], f16, tag="ybank",
                                                name="ybank")
                    yb = ybanks[b]
                    nc.scalar.copy(yb[:, ps * B_LOC: (ps + 1) * B_LOC], pyps[:])
                    if ps == TB - 1 or pk == NT - 1:
                        nsl = ps + 1
                        nc.sync.dma_start(out_d[:, b * TB: b * TB + nsl, :],
                                          yb[:, 0: nsl * B_LOC])

            for k in range(NT):
                y_ps = pypool.tile([P, B_LOC], f32, tag="yps")
                for g in range(NG):
                    w = w_cur[g]
                    # phase: s2 -> m2 -> q -> q2
                    s2 = spool.tile([P, GW], f16, tag=f"s2_{g}")
                    nc.scalar.activation(s2[:], w, Square)
                    m2 = pmpool.tile([P, GW], f32, tag=f"m2_{g}")
                    nc.tensor.matmul(m2[:], wt["P2T"][:], s2[:],
                                     start=True, stop=True)
                    q = spool.tile([P, GW], f16, tag=f"q_{g}")
                    nc.scalar.activation(q[:], m2[:], Sin,
                                         bias=qb_t[:], scale=ghs_t[:])
                    # rotation products (r1 first so chain MMs can start early)
                    r1 = spool.tile([P, GW], f16, tag=f"r1_{g}")
                    nc.vector.tensor_tensor(r1[:], w, q[:], mult)
                    q2 = spool.tile([P, GW], f16, tag=f"q2_{g}")
                    nc.vector.stream_shuffle(q2[:], q[:], pairswap)
                    r2 = spool.tile([P, GW], f16, tag=f"r2_{g}")
                    nc.vector.tensor_tensor(r2[:], w, q2[:], mult)
                    if k < NT - 1:
                        # chain w_{k+1} (before Y so w_ps completes earliest)
                        w_ps = pwpool.tile([P, GW], f32, tag=f"wps_{g}")
                        nc.tensor.matmul(w_ps[:], wt["W1hi"][:], r1[:], start=True, stop=False)
                        nc.tensor.matmul(w_ps[:], wt["W1lo"][:], r1[:], start=False, stop=False)
                        nc.tensor.matmul(w_ps[:], wt["W2hi"][:], r2[:], start=False, stop=False)
                        nc.tensor.matmul(w_ps[:], wt["W2lo"][:], r2[:], start=False, stop=True)
                        w_cur[g] = w_ps[:]
                    # output y_{k+1}
                    ysl = y_ps[:, g * GW: (g + 1) * GW]
                    nc.tensor.matmul(ysl, wt["Y1"][:], r1[:], start=True, stop=False)
                    nc.tensor.matmul(ysl, wt["Y2"][:], r2[:], start=False, stop=True)
                stage_pending()
                pend.append((k, y_ps))
            stage_pending()

    nc.compile()
    return nc


def _get_compiled():
    if "nc" not in _CACHE:
        _CACHE["nc"] = _build_nc()
    return _CACHE["nc"]


def _run(host, trace=False, tmpdir=None):
    from concourse.bass_utils import run_bass_kernel_spmd

    nc = _get_compiled()
    in_maps = []
    for i in range(N_CORES):
        sl = slice(i * B_LOC, (i + 1) * B_LOC)
        m = {name: host[name] for name in
             ("W1hi", "W1lo", "W2hi", "W2lo", "Y1", "Y2", "P2T",
              "ghscale", "qbias")}
        m["w0"] = np.ascontiguousarray(host["w0"][:, sl])
        in_maps.append(m)
    res = run_bass_kernel_spmd(nc, in_maps, list(range(N_CORES)), trace=trace,
                               tmpdir=tmpdir)
    full = np.empty((EVAL_PTS, BATCH, MODES, 2), dtype=np.float32)
    # t = 0: host passthrough
    full[0] = host["y0f"].T.reshape(BATCH, MODES, 2)
    for i in range(N_CORES):
        sl = slice(i * B_LOC, (i + 1) * B_LOC)
        arr = res.results[i]["out"].astype(np.float32)  # (128, 199, 128) part-major
        full[1:, sl, :, :] = arr.transpose(1, 2, 0).reshape(EVAL_PTS - 1, B_LOC, MODES, 2)
    return full, res


def kernel(A0, params, omega, kappa, nonlinearity):
    A0 = np.asarray(A0, dtype=np.float32)
    params = np.asarray(params, dtype=np.float32)
    omega = np.asarray(omega, dtype=np.float32)
    kappa = np.asarray(kappa, dtype=np.float32)
    nonlinearity = np.asarray(nonlinearity, dtype=np.float32)

    host = _host_precompute(A0, params, omega, kappa, nonlinearity)
    full, _ = _run(host, trace=False)
    return full


# revision 5
# speedup vs baseline: 1.0244x; 1.0244x over previous
"""Trainium2 kernel v2 for the nn_Circuit coupled-mode ODE problem.

Math: dA/dt = i*diag(omega + gamma*|A|^2) A + T2 A, t in [0,2], 200 samples,
A is (1024 batch, 64 modes) complex padded with ones.  Strang splitting with
the linear part exact (host matrix exponentials) and the nonlinear part an
exact per-pair phase rotation, phase computed IN-STEP from |A|^2 (lagged
phases fail: the stiff ~288i eigenvalue makes |A|^2 swing ~25% per step).

Device formulation (interleaved re/im layout, partitions p=2j+c):
  state w_k (fp16 SBUF) at the rotation midpoint.  Per step:
    s2  = w (.) w                       (V, fp16 2x)
    m2  = P2 @ s2                       (PE; pair-sum, duplicated per pair)
    q   = sin(gh*m2 + bias)             (ACT; bias pi/2 even / 0 odd -> [cos;sin])
    q2  = pairswap(q)                   (V stream_shuffle -> [sin;cos])
    r1  = w (.) q,  r2 = w (.) q2       (V, fp16 2x)
    y_{k+1} = Y1 r1 + Y2 r2             (PE -> PSUM y-bank, DMA'd 4 steps at a time)
    w_ps    = W1 r1 + W2 r2             (PE; W split hi+lo fp16 for accuracy)
    w_{k+1} = copy(w_ps)                (V, PSUM->SBUF fp16 2x)
  where W1 = E(h) M1, W2 = E(h) M2, Y1 = E(h/2) M1, Y2 = E(h/2) M2 fold the
  complex-multiply pair-mixing (M1: re' = r1[2j]-r1[2j+1] -> even rows,
  M2: im' = r2[2j]+r2[2j+1] -> odd rows) into the matmul weights.

Two 64-column batch groups run the chain staggered so PE/V/ACT overlap across
groups; all weights fp16 (FWL fast weight load) so matmuls are single-pass.

Sharding: pure data parallel, batch 1024 = 8 cores x 128.
"""

import numpy as np

MODES = 64
INPUT_MODES = 48
BATCH = 1024
EVAL_PTS = 200
EPS = 1e-8
N_CORES = 8
B_LOC = BATCH // N_CORES  # 128
NT = EVAL_PTS - 1  # 199 intervals
DT = 2.0 / NT
NG = 2              # column groups per core
GW = B_LOC // NG    # 64 columns per group
TB = 8              # output time-steps batched per y-bank DMA

_CACHE = {}


# ---------------------------------------------------------------------------
# host-side math
# ---------------------------------------------------------------------------

def _t2_like_reference(params, omega, kappa):
    """Reproduce the reference's float32 jax computation of T2 exactly."""
    import jax

    try:
        cpu = jax.devices("cpu")[0]
    except Exception:
        cpu = None

    import contextlib

    ctx = jax.default_device(cpu) if cpu is not None else contextlib.nullcontext()
    with ctx:
        import jax.numpy as jnp

        n = MODES
        p = jnp.asarray(params, dtype=jnp.float32)
        n_off = n * (n - 1) // 2
        iu = jnp.triu_indices(n, 1)
        off = p[:n_off] + 1j * p[n_off:2 * n_off]
        H = jnp.zeros((n, n), dtype=jnp.complex64).at[iu].set(off.astype(jnp.complex64))
        H = H + H.conj().T
        d = p[2 * n_off:]
        diag = jnp.concatenate([d, -jnp.sum(d, keepdims=True)])
        H = H + jnp.diag(diag.astype(jnp.complex64))
        U = jax.scipy.linalg.expm(1j * H)
        I = jnp.eye(n, dtype=jnp.complex64)
        M = U.T @ U
        mix = M @ jnp.linalg.inv(I - M + EPS * I)
        T2 = -jnp.asarray(kappa, dtype=jnp.float32) * (
            0.5 * jnp.eye(n, dtype=jnp.float32) + mix
        )
        T2_re = np.asarray(jnp.real(T2), dtype=np.float32)
        T2_im = np.asarray(jnp.imag(T2), dtype=np.float32)
    return T2_re, T2_im


def _expm(M):
    w, V = np.linalg.eig(M)
    return (V * np.exp(w)) @ np.linalg.inv(V)


def _big_il(C):
    """Complex (64,64) -> real (128,128) operator in the interleaved re/im basis."""
    A = np.zeros((2 * MODES, 2 * MODES), dtype=np.float64)
    Cr, Ci = C.real, C.imag
    A[0::2, 0::2] = Cr
    A[0::2, 1::2] = -Ci
    A[1::2, 0::2] = Ci
    A[1::2, 1::2] = Cr
    return A


def _host_precompute(A0, params, omega, kappa, nonlinearity):
    T2_re, T2_im = _t2_like_reference(params, omega, kappa)
    L = T2_re.astype(np.float64) + 1j * T2_im.astype(np.float64)
    L = L + 1j * np.diag(omega.astype(np.float64))

    A1 = _big_il(_expm(L * DT))         # full-step propagator E(h)
    A2 = _big_il(_expm(L * (DT / 2)))   # half-step propagator E(h/2)

    # pair-mixing matrices (signs included)
    M1 = np.zeros((128, 128))
    M2 = np.zeros((128, 128))
    for j in range(MODES):
        M1[2 * j, 2 * j] = 1.0
        M1[2 * j, 2 * j + 1] = -1.0
        M2[2 * j + 1, 2 * j] = 1.0
        M2[2 * j + 1, 2 * j + 1] = 1.0

    W1 = A1 @ M1
    W2 = A1 @ M2
    Y1 = A2 @ M1
    Y2 = A2 @ M2

    def lhsT16(Wm):
        return np.ascontiguousarray(Wm.T, dtype=np.float16)

    def split16(Wm):
        hi = Wm.astype(np.float16).astype(np.float64)
        lo = Wm - hi
        return lhsT16(hi), lhsT16(lo)


    P2 = np.zeros((128, 128))
    for j in range(MODES):
        P2[2 * j, 2 * j] = P2[2 * j, 2 * j + 1] = 1.0
        P2[2 * j + 1, 2 * j] = P2[2 * j + 1, 2 * j + 1] = 1.0

    # initial state, interleaved mode-major: (128, BATCH)
    y0 = np.zeros((2 * MODES, BATCH), dtype=np.float64)
    y0[0:2 * INPUT_MODES:2, :] = A0[:, :, 0].astype(np.float64).T
    y0[1:2 * INPUT_MODES:2, :] = A0[:, :, 1].astype(np.float64).T
    y0[2 * INPUT_MODES::2, :] = 1.0
    w0 = (A2 @ y0).astype(np.float16)
    y0f = y0.astype(np.float32)

    gh = nonlinearity.astype(np.float64) * DT
    ghscale = np.repeat(gh, 2).astype(np.float32).reshape(128, 1)
    qbias = np.tile([np.pi / 2, 0.0], MODES).astype(np.float32).reshape(128, 1)

    W1hi, W1lo = split16(W1)
    W2hi, W2lo = split16(W2)
    return dict(
        W1hi=W1hi, W1lo=W1lo, W2hi=W2hi, W2lo=W2lo,
        Y1=lhsT16(Y1), Y2=lhsT16(Y2), P2T=lhsT16(P2.T),
        ghscale=ghscale, qbias=qbias, w0=w0, y0f=y0f,
    )


# ---------------------------------------------------------------------------
# device kernel
# ---------------------------------------------------------------------------

def _build_nc():
    import concourse.bass as bass
    import concourse.bacc as bacc
    import concourse.tile as tile
    import concourse.mybir as mybir

    f32 = mybir.dt.float32
    f16 = mybir.dt.float16
    Sin = mybir.ActivationFunctionType.Sin
    Square = mybir.ActivationFunctionType.Square
    mult = mybir.AluOpType.mult
    P = 128
    pairswap = [i ^ 1 for i in range(32)]

    nc = bacc.Bacc("TRN2", target_bir_lowering=False, debug=False,
                   num_devices=N_CORES)

    wts = {}
    for name in ("W1hi", "W1lo", "W2hi", "W2lo", "Y1", "Y2", "P2T"):
        wts[name] = nc.dram_tensor(name, [P, P], f16, kind="ExternalInput").ap()
    ghscale_d = nc.dram_tensor("ghscale", [P, 1], f32, kind="ExternalInput").ap()
    qbias_d = nc.dram_tensor("qbias", [P, 1], f32, kind="ExternalInput").ap()
    w0_d = nc.dram_tensor("w0", [P, B_LOC], f16, kind="ExternalInput").ap()
    # output t=1..199 (t=0 is assembled on the host), time-major per partition
    out_d = nc.dram_tensor("out", [P, EVAL_PTS - 1, B_LOC], f16,
                           kind="ExternalOutput").ap()

    with tile.TileContext(nc) as tc:
        with (
            tc.tile_pool(name="const", bufs=1) as cpool,
            tc.tile_pool(name="st", bufs=3) as spool,       # per-group SBUF work tiles
            tc.tile_pool(name="pw", bufs=2, space="PSUM") as pwpool,   # w_ps
            tc.tile_pool(name="pm", bufs=1, space="PSUM") as pmpool,   # m2
            tc.tile_pool(name="py", bufs=2, space="PSUM") as pypool,   # y step tiles
            tc.tile_pool(name="yb", bufs=2) as ybpool,                 # SBUF y banks
        ):
            wt = {}
            for name in ("W1hi", "W1lo", "W2hi", "W2lo", "Y1", "Y2", "P2T"):
                wt[name] = cpool.tile([P, P], f16, tag=name, name=name)
                nc.sync.dma_start(wt[name][:], wts[name][:])
            ghs_t = cpool.tile([P, 1], f32, tag="ghs")
            qb_t = cpool.tile([P, 1], f32, tag="qb")
            nc.sync.dma_start(ghs_t[:], ghscale_d[:])
            nc.sync.dma_start(qb_t[:], qbias_d[:])

            # initial chain state per group (SBUF; later steps read w from PSUM)
            w0_t = cpool.tile([P, B_LOC], f16, tag="w0t")
            nc.sync.dma_start(w0_t[:], w0_d[:])
            w_cur = [w0_t[:, g * GW:(g + 1) * GW] for g in range(NG)]

            ybanks = {}   # bank index -> (tile, pending y_ps slices)
            pend = []     # deferred (k, y_ps) staging copies

            def stage_pending():
                # emit the previous step's y staging AFTER this step's SINs so
                # the ACT queue never blocks the phase chain
                while pend:
                    pk, pyps = pend.pop(0)
                    b, ps = divmod(pk, TB)
                    if ps == 0:
                        ybanks[b] = ybpool.tile([P, TB * B_LOC], f16, tag="ybank",
                                                name="ybank")
                    yb = ybanks[b]
                    nc.vector.tensor_copy(yb[:, ps * B_LOC: (ps + 1) * B_LOC],
                                          pyps[:])
                    if ps == TB - 1 or pk == NT - 1:
                        nsl = ps + 1
                        nc.sync.dma_start(out_d[:, b * TB: b * TB + nsl, :],
                                          yb[:, 0: nsl * B_LOC])

            for k in range(NT):
                y_ps = pypool.tile([P, B_LOC], f32, tag="yps")
                for g in range(NG):
                    w = w_cur[g]
                    # phase: s2 -> m2 -> q -> q2
                    s2 = spool.tile([P, GW], f16, tag=f"s2_{g}")
                    nc.scalar.activation(s2[:], w, Square)
                    m2 = pmpool.tile([P, GW], f32, tag=f"m2_{g}")
                    nc.tensor.matmul(m2[:], wt["P2T"][:], s2[:],
                                     start=True, stop=True)
                    q = spool.tile([P, GW], f16, tag=f"q_{g}")
                    nc.scalar.activation(q[:], m2[:], Sin,
                                         bias=qb_t[:], scale=ghs_t[:])
                    # rotation products (r1 first so chain MMs can start early)
                    r1 = spool.tile([P, GW], f16, tag=f"r1_{g}")
                    nc.vector.tensor_tensor(r1[:], w, q[:], mult)
                    q2 = spool.tile([P, GW], f16, tag=f"q2_{g}")
                    nc.vector.stream_shuffle(q2[:], q[:], pairswap)
                    r2 = spool.tile([P, GW], f16, tag=f"r2_{g}")
                    nc.vector.tensor_tensor(r2[:], w, q2[:], mult)
                    if k < NT - 1:
                        # chain w_{k+1} (before Y so w_ps completes earliest)
                        w_ps = pwpool.tile([P, GW], f32, tag=f"wps_{g}")
                        nc.tensor.matmul(w_ps[:], wt["W1hi"][:], r1[:], start=True, stop=False)
                        nc.tensor.matmul(w_ps[:], wt["W1lo"][:], r1[:], start=False, stop=False)
                        nc.tensor.matmul(w_ps[:], wt["W2hi"][:], r2[:], start=False, stop=False)
                        nc.tensor.matmul(w_ps[:], wt["W2lo"][:], r2[:], start=False, stop=True)
                        w_cur[g] = w_ps[:]
                    # output y_{k+1}
                    ysl = y_ps[:, g * GW: (g + 1) * GW]
                    nc.tensor.matmul(ysl, wt["Y1"][:], r1[:], start=True, stop=False)
                    nc.tensor.matmul(ysl, wt["Y2"][:], r2[:], start=False, stop=True)
                stage_pending()
                pend.append((k, y_ps))
            stage_pending()

    nc.compile()
    return nc


def _get_compiled():
    if "nc" not in _CACHE:
        _CACHE["nc"] = _build_nc()
    return _CACHE["nc"]


def _run(host, trace=False, tmpdir=None):
    from concourse.bass_utils import run_bass_kernel_spmd

    nc = _get_compiled()
    in_maps = []
    for i in range(N_CORES):
        sl = slice(i * B_LOC, (i + 1) * B_LOC)
        m = {name: host[name] for name in
             ("W1hi", "W1lo", "W2hi", "W2lo", "Y1", "Y2", "P2T",
              "ghscale", "qbias")}
        m["w0"] = np.ascontiguousarray(host["w0"][:, sl])
        in_maps.append(m)
    res = run_bass_kernel_spmd(nc, in_maps, list(range(N_CORES)), trace=trace,
                               tmpdir=tmpdir)
    full = np.empty((EVAL_PTS, BATCH, MODES, 2), dtype=np.float32)
    # t = 0: host passthrough
    full[0] = host["y0f"].T.reshape(BATCH, MODES, 2)
    for i in range(N_CORES):
        sl = slice(i * B_LOC, (i + 1) * B_LOC)
        arr = res.results[i]["out"].astype(np.float32)  # (128, 199, 128) part-major
        full[1:, sl, :, :] = arr.transpose(1, 2, 0).reshape(EVAL_PTS - 1, B_LOC, MODES, 2)
    return full, res


def kernel(A0, params, omega, kappa, nonlinearity):
    A0 = np.asarray(A0, dtype=np.float32)
    params = np.asarray(params, dtype=np.float32)
    omega = np.asarray(omega, dtype=np.float32)
    kappa = np.asarray(kappa, dtype=np.float32)
    nonlinearity = np.asarray(nonlinearity, dtype=np.float32)

    host = _host_precompute(A0, params, omega, kappa, nonlinearity)
    full, _ = _run(host, trace=False)
    return full
